# revision 30
# baseline (speedup 1.0000x reference)
"""Trainium2 Bass kernel for nn_CompositionBlock (gnn_message_passing).

Reference semantics (per batch b, S=2048 tokens, T=128 dims):
    h        = tanh(token)                               # [S, T]
    val[s,t] = sum_pq token[s,p] W[t,p,q] h[s,q] + b_comp[t]
    act      = tanh(val)
    delta    = w_red[s] * (act[s,t] - tanh(b_comp)[t])
    out[i,t] = sum_s w_red[s]*tanh(b_comp)[t] + b_red
               + sum_{s: heads[s]==i} delta[s,t]

Sharding: data-parallel over batch B=8 -> one batch per NeuronCore; W and
the small vectors replicated. No collectives.

Device algorithm per core (all matmuls fp16 in / f32 psum accum):
  MM1 (PE):  A_t[q, s] = W_t[p,q].T @ tokenT[p, s]   (per t, s-group of 512)
  TT  (VE):  Z_t[q, s] = A_t * hT[q, s]  (x=3/4 of t-pairs via a ScalarE
             fp16 downcast so the DVE multiply runs 2x packed; 1/4 read
             PSUM directly at 1x -- balances ACT vs DVE busy time)
  MM2 (PE):  q-reduction via 32-wide column tiling of the PE array: t's
             are processed in strided quads (m, m+32, m+64, m+96) so four
             reduction matmuls (lhsT = staircase slice with a ones column
             at t%32, out = V[32j:32j+32]) execute concurrently in
             different PE column groups -- ~4x cheaper than a full-array
             staircase reduction. MM2 emission trails MM1 by LAG t-pairs
             (software pipelining) so the PE queue never blocks on Z.
  ACT:       actT = tanh(valT + b_comp[t])  (per-partition bias)
  deltaT = actT - tanh(b_comp)[t];  DMA-xbar transpose -> delta[j, t];
  delta_w = w_red[j] * delta.
  one-hot (VE): MT[j,i] = (heads[j] == i) via is_equal vs iota row.
  MM3 (PE):  outT[t,i] += delta_w_j.T @ MT_j over j-tiles; += base[t]; DMA.
base[t] = sum(w_red)*tanh(b_comp)[t] + b_red and tanh(b_comp) are computed
on the host (keeps gpsimd's slow ext-isa partition reduce off device).
W is staged fp16 and loaded in 512-col pieces ordered by first use of the
strided t-sequence. Host transposes outT -> out per batch at gather time.
"""

import os
from contextlib import ExitStack

import numpy as np

import concourse.bass as bass
import concourse.tile as tile
from concourse import bacc, mybir
from concourse.bass_utils import run_bass_kernel_spmd

B, S, T = 8, 2048, 128
P = 128
N_CORES = 8
NST = S // P      # 16 s-tiles of 128
NSG = S // 512    # 4 s-groups of 512
F32 = mybir.dt.float32
F16 = mybir.dt.float16
I32 = mybir.dt.int32
AF = mybir.ActivationFunctionType
ALU = mybir.AluOpType

_NC_CACHE = {}


def build_nc():
    nc = bacc.Bacc("TRN2", target_bir_lowering=False, debug=False,
                   num_devices=N_CORES)

    tokT_d = nc.dram_tensor("tokT", [T, S], F16, kind="ExternalInput").ap()
    w_ptq_d = nc.dram_tensor("w_ptq", [P, T * T], F16, kind="ExternalInput").ap()
    bcompT_d = nc.dram_tensor("bcompT", [T, 1], F32, kind="ExternalInput").ap()
    wred_d = nc.dram_tensor("wred", [P, NST], F32, kind="ExternalInput").ap()
    heads_d = nc.dram_tensor("heads", [P, NST], I32, kind="ExternalInput").ap()
    basevT_d = nc.dram_tensor("basevT", [T, 1], F32,
                              kind="ExternalInput").ap()
    baseT_d = nc.dram_tensor("baseT", [T, 1], F32, kind="ExternalInput").ap()
    iota_d = nc.dram_tensor("iota", [1, S], F16, kind="ExternalInput").ap()
    outT_d = nc.dram_tensor("outT", [T, S], F32, kind="ExternalOutput").ap()

    with tile.TileContext(nc) as tc:
        _body(tc, tokT_d, w_ptq_d, bcompT_d, wred_d, heads_d, basevT_d,
              baseT_d, iota_d, outT_d)
    nc.compile()
    return nc


def _body(tc, tokT_d, w_ptq_d, bcompT_d, wred_d, heads_d, basevT_d, baseT_d,
          iota_d, outT_d):
    nc = tc.nc
    with ExitStack() as ctx:
        const = ctx.enter_context(tc.tile_pool(name="const", bufs=1))
        zpool = ctx.enter_context(tc.tile_pool(name="zpool", bufs=4))
        zbigp = ctx.enter_context(tc.tile_pool(name="zbigp", bufs=3))
        a16p = ctx.enter_context(tc.tile_pool(name="a16p", bufs=2))
        h6p = ctx.enter_context(tc.tile_pool(name="h6p", bufs=2))
        spool = ctx.enter_context(tc.tile_pool(name="spool", bufs=2))
        djp = ctx.enter_context(tc.tile_pool(name="djp", bufs=3))
        dwp = ctx.enter_context(tc.tile_pool(name="dwp", bufs=1))
        mtp = ctx.enter_context(tc.tile_pool(name="mtp", bufs=1))

        # ---- constants / inputs ----
        # Separate tiles per chunk so matmul deps release as each cast-DMA
        # lands (whole-tile deps would stall PE on the full 9MB load).
        # Queue order: tokT_0, W_0 first -> first MM1 starts after ~2 chunks.
        # staircase for 32-col-tiled reduction: Q32[:, 31] = 1, else 0;
        # E32_r = Q32[:, 31-r : 63-r] has its ones-column at position r.
        # Built first: MM2 needs it and gpsimd memsets queue behind any
        # DMA-descriptor work emitted earlier.
        Q32 = const.tile([P, 63], F16)
        nc.gpsimd.memset(Q32[:], 0.0)
        nc.gpsimd.memset(Q32[:, 31: 32], 1.0)

        tokTs = [const.tile([P, 512], F16, tag=f"tokT{g}", name=f"tokT{g}")
                 for g in range(NSG)]
        w_tiles = [const.tile([P, 2048], F16, tag=f"w{wc}", name=f"w{wc}")
                   for wc in range(8)]

        def load_w_pieces(wcs, piece, eng):
            ps = slice(512 * piece, 512 * (piece + 1))
            for wc in wcs:
                eng.dma_start(
                    out=w_tiles[wc][:, ps],
                    in_=w_ptq_d[:, 2048 * wc + 512 * piece:
                                2048 * wc + 512 * (piece + 1)])

        # Load order: tokT0 + the W pieces the first m-quads need lead the
        # sync (HWDGE) queue so compute starts without waiting on SWDGE
        # descriptor generation; the rest streams on the gpsimd queue in
        # strided-need order (tiles {0,2,4,6} serve t%32 in [0,16),
        # {1,3,5,7} serve m 16-31).
        nc.sync.dma_start(out=tokTs[0][:], in_=tokT_d[:, 0:512])
        load_w_pieces((0, 2, 4, 6), 0, nc.sync)
        for g in range(1, NSG):
            nc.gpsimd.dma_start(out=tokTs[g][:],
                                in_=tokT_d[:, 512 * g: 512 * (g + 1)])
        for piece in range(1, 4):
            load_w_pieces((0, 2, 4, 6), piece, nc.gpsimd)
        for piece in range(4):
            load_w_pieces((1, 3, 5, 7), piece, nc.gpsimd)
        # hT2 tanh builds are deferred into the main loop (start of each
        # group) so they don't block the ScalarE queue at startup.
        hT2s = [const.tile([P, 1024], F16, tag=f"hT2_{g}", name=f"hT2_{g}")
                for g in range(NSG)]
        iota_sb = const.tile([P, S], F16)
        nc.sync.dma_start(out=iota_sb[:], in_=iota_d[0:1, :].to_broadcast((P, S)))
        wred_sb = const.tile([P, NST], F32)
        nc.sync.dma_start(out=wred_sb[:], in_=wred_d[:])
        heads_sb = const.tile([P, NST], I32)
        nc.sync.dma_start(out=heads_sb[:], in_=heads_d[:])
        headsF = const.tile([P, NST], F32)
        nc.vector.tensor_copy(headsF[:], heads_sb[:])
        bcompT_sb = const.tile([T, 1], F32)
        nc.sync.dma_start(out=bcompT_sb[:], in_=bcompT_d[:])
        # basev = tanh(b_comp) and baseT = sum(w_red)*basev + b_red are
        # host-precomputed: keeps the slow gpsimd partition_all_reduce (and
        # its ~10us ext-isa library load) off the critical path.
        basevT = const.tile([T, 1], F32)
        nc.sync.dma_start(out=basevT[:], in_=basevT_d[:])
        baseT = const.tile([P, 1], F32)
        nc.sync.dma_start(out=baseT[:], in_=baseT_d[:])



        # ---- main loop: s-groups of 512, t processed in strided pairs
        # (m, m+32) / (m+64, m+96) so the reduction matmuls of consecutive
        # Z tiles land in different 32-wide PE column groups and execute
        # concurrently (4-way col tiling of the PE array). ----
        dws = []
        with tc.tile_pool(name="psumA", bufs=3, space="PSUM") as psumA, \
             tc.tile_pool(name="psumV", bufs=2, space="PSUM") as psumV:
            LAG = 7  # MM2 trails MM1 by LAG tps so PE never waits on Z
            TPG = T // 2  # tps per s-group
            NTP = NSG * TPG
            mm2q = []
            Vs = [None] * NSG
            hT6s = [None] * NSG

            def emit_group_tail(g):
                # after the last MM2 of group g: tanh, delta, transpose, scale
                actT = spool.tile([P, 512], F16, tag="actT", name="actT")
                nc.scalar.activation(actT[:], Vs[g][:], AF.Tanh,
                                     bias=bcompT_sb[:])
                dT = spool.tile([P, 512], F16, tag="dT", name="dT")
                nc.vector.tensor_scalar_sub(dT[:], actT[:], basevT[:])
                for k in range(4):
                    j = 4 * g + k
                    dj = djp.tile([P, P], F16, tag="dj", name="dj")
                    nc.sync.dma_start_transpose(out=dj[:],
                                                in_=dT[:, P * k: P * (k + 1)])
                    dw_j = dwp.tile([P, P], F16, tag=f"dw{j}", name=f"dw{j}")
                    nc.vector.tensor_scalar_mul(dw_j[:], dj[:],
                                                wred_sb[:, j: j + 1])
                    dws.append(dw_j)

            for gtp in range(NTP + LAG + 1):
                if gtp < NTP:
                    g, tp = gtp // TPG, gtp % TPG
                    if tp == 0:
                        hT2 = hT2s[g]
                        nc.scalar.activation(hT2[:, 0:512], tokTs[g][:],
                                             AF.Tanh)
                        nc.scalar.activation(hT2[:, 512:1024], tokTs[g][:],
                                             AF.Tanh)
                        # hT replicated x6 so one FD=3072 DVE multiply
                        # covers three t-pairs.
                        hT6 = h6p.tile([P, 3072], F16, tag="hT6", name="hT6")
                        for rep in range(3):
                            nc.vector.tensor_copy(
                                hT6[:, 1024 * rep: 1024 * (rep + 1)],
                                hT2[:])
                        hT6s[g] = hT6
                        Vs[g] = psumV.tile([P, 512], F32, space="PSUM",
                                           tag="V", name="V")
                    m, half = tp // 2, tp % 2
                    t0 = m + 64 * half
                    t1 = t0 + 32
                    A = psumA.tile([P, 1024], F32, space="PSUM", tag="A",
                                   name="A")
                    w_t0 = w_tiles[t0 // 16][:,
                                             T * (t0 % 16): T * (t0 % 16 + 1)]
                    w_t1 = w_tiles[t1 // 16][:,
                                             T * (t1 % 16): T * (t1 % 16 + 1)]
                    nc.tensor.matmul(A[:, 0:512], lhsT=w_t0,
                                     rhs=tokTs[g][:], start=True, stop=True)
                    nc.tensor.matmul(A[:, 512:1024], lhsT=w_t1,
                                     rhs=tokTs[g][:], start=True, stop=True)
                    c = tp % 4
                    if c != 3:
                        # ScalarE downcast to fp16 SBUF (2x-packed DVE); the
                        # three downcasts of a cycle land in one A16 tile so
                        # a single FD=3072 DVE multiply covers all three
                        # t-pairs (amortizes the fixed per-op DVE cost).
                        if c == 0:
                            A16 = a16p.tile([P, 3072], F16, tag="A16",
                                            name="A16")
                            Zb = zbigp.tile([P, 3072], F16, tag="Zb",
                                            name="Zb")
                        nc.scalar.activation(A16[:, 1024 * c: 1024 * (c + 1)],
                                             A[:], AF.Copy)
                        if c == 2:
                            nc.vector.tensor_tensor(out=Zb[:], in0=A16[:],
                                                    in1=hT6s[g][:],
                                                    op=ALU.mult)
                        mm2q.append((g, m, t0, t1, Zb, 1024 * c))
                    else:
                        Z = zpool.tile([P, 1024], F16, tag="Z", name="Z")
                        nc.vector.tensor_tensor(out=Z[:], in0=A[:],
                                                in1=hT2s[g][:], op=ALU.mult)
                        mm2q.append((g, m, t0, t1, Z, 0))
                # 32-col-tiled q-reduction: row r of col group j gets
                # sum_q Z_t with t = 32j + r. Emit as full quads (all 4
                # col groups back-to-back) so the four matmuls execute
                # concurrently in one PE slot. The lag queue is global so
                # the pipeline never refills at s-group boundaries.
                if gtp > LAG and (gtp - LAG) % 2 == 1:
                    for k in (gtp - LAG - 1, gtp - LAG):
                        g, m, t0, t1, Z, off = mm2q[k]
                        for t, zs in ((t0, slice(off, off + 512)),
                                      (t1, slice(off + 512, off + 1024))):
                            j, r = t // 32, t % 32
                            nc.tensor.matmul(Vs[g][32 * j: 32 * (j + 1), :],
                                             lhsT=Q32[:, 31 - r: 63 - r],
                                             rhs=Z[:, zs],
                                             start=(m == 0),
                                             stop=(m == 31),
                                             tile_position=(0, 32 * j),
                                             skip_group_check=True)
                        if m == 31 and t1 == 127:
                            emit_group_tail(g)

        # ---- one-hot scatter matrices: MT[j, i] = (iota[i] == head[j]).
        # Built late so they fill DVE slack near the main loop's tail. ----
        mts = []
        for j in range(NST):
            mt_j = mtp.tile([P, S], F16, tag=f"mt{j}", name=f"mt{j}")
            nc.vector.tensor_scalar(out=mt_j[:], in0=iota_sb[:],
                                    scalar1=headsF[:, j: j + 1], scalar2=None,
                                    op0=ALU.is_equal)
            mts.append(mt_j)

        # ---- scatter: outT[t, i] = sum_j delta_w[j, t] * MT[j, i] + base ----
        # c-outer so each output chunk's add + store overlaps the next
        # chunk's matmuls.
        outT_sb = const.tile([P, S], F32)
        with tc.tile_pool(name="psumO", bufs=1, space="PSUM") as psumO:
            for c in range(4):
                OT = psumO.tile([P, 512], F32, space="PSUM", tag=f"OT{c}",
                                name=f"OT{c}")
                for j in range(NST):
                    nc.tensor.matmul(OT[:], lhsT=dws[j][:],
                                     rhs=mts[j][:, 512 * c: 512 * (c + 1)],
                                     start=(j == 0), stop=(j == NST - 1))
                cs = slice(512 * c, 512 * (c + 1))
                nc.vector.tensor_scalar_add(outT_sb[:, cs], OT[:], baseT[:])
                nc.sync.dma_start(out=outT_d[:, cs], in_=outT_sb[:, cs])


def _prep_inputs(token_embeddings, dep_heads, W_comp, b_comp, w_red, b_red):
    """Host-side sharding + layout prep. One in_map per core (= per batch)."""
    token = np.asarray(token_embeddings, np.float32).astype(np.float16)
    heads = np.asarray(dep_heads, np.int32)
    W = np.asarray(W_comp, np.float32).astype(np.float16)
    w_ptq = np.ascontiguousarray(W.transpose(1, 0, 2).reshape(P, T * T))
    bcomp = np.asarray(b_comp, np.float32)
    bcompT = np.ascontiguousarray(bcomp.reshape(T, 1))
    wred_flat = np.asarray(w_red, np.float32).reshape(S)
    wred = np.ascontiguousarray(wred_flat.reshape(NST, P).T)
    bred = float(np.asarray(b_red, np.float32).reshape(1)[0])
    basev = np.tanh(bcomp.astype(np.float64))
    baseT = (wred_flat.astype(np.float64).sum() * basev + bred)
    basevT = np.ascontiguousarray(basev.astype(np.float32).reshape(T, 1))
    baseT = np.ascontiguousarray(baseT.astype(np.float32).reshape(T, 1))
    iota = np.arange(S, dtype=np.float16).reshape(1, S)

    in_maps = []
    for b in range(B):
        in_maps.append({
            "tokT": np.ascontiguousarray(token[b].T),
            "w_ptq": w_ptq,
            "bcompT": bcompT,
            "wred": wred,
            "heads": np.ascontiguousarray(heads[b].reshape(NST, P).T),
            "basevT": basevT,
            "baseT": baseT,
            "iota": iota,
        })
    return in_maps


def kernel(**inputs):
    if "nc" not in _NC_CACHE:
        _NC_CACHE["nc"] = build_nc()
    nc = _NC_CACHE["nc"]
    in_maps = _prep_inputs(
        inputs["token_embeddings"], inputs["dep_heads"], inputs["W_comp"],
        inputs["b_comp"], inputs["w_red"], inputs["b_red"])
    res = run_bass_kernel_spmd(nc, in_maps, core_ids=list(range(N_CORES)))
    out = np.empty((B, S, T), np.float32)
    for b in range(B):
        out[b] = res.results[b]["outT"].T
    return out



# revision 31
# speedup vs baseline: 1.0123x; 1.0123x over previous
"""Trainium2 Bass kernel for nn_CompositionBlock (gnn_message_passing).

Reference semantics (per batch b, S=2048 tokens, T=128 dims):
    h        = tanh(token)                               # [S, T]
    val[s,t] = sum_pq token[s,p] W[t,p,q] h[s,q] + b_comp[t]
    act      = tanh(val)
    delta    = w_red[s] * (act[s,t] - tanh(b_comp)[t])
    out[i,t] = sum_s w_red[s]*tanh(b_comp)[t] + b_red
               + sum_{s: heads[s]==i} delta[s,t]

Sharding: data-parallel over batch B=8 -> one batch per NeuronCore; W and
the small vectors replicated. No collectives.

Device algorithm per core (all matmuls fp16 in / f32 psum accum):
  MM1 (PE):  A_t[q, s] = W_t[p,q].T @ tokenT[p, s]   (per t, s-group of 512)
  TT  (VE):  Z_t[q, s] = A_t * hT[q, s]  (x=3/4 of t-pairs via a ScalarE
             fp16 downcast so the DVE multiply runs 2x packed; 1/4 read
             PSUM directly at 1x -- balances ACT vs DVE busy time)
  MM2 (PE):  q-reduction via 32-wide column tiling of the PE array: t's
             are processed in strided quads (m, m+32, m+64, m+96) so four
             reduction matmuls (lhsT = staircase slice with a ones column
             at t%32, out = V[32j:32j+32]) execute concurrently in
             different PE column groups -- ~4x cheaper than a full-array
             staircase reduction. MM2 emission trails MM1 by LAG t-pairs
             (software pipelining) so the PE queue never blocks on Z.
  ACT:       actT = tanh(valT + b_comp[t])  (per-partition bias)
  deltaT = actT - tanh(b_comp)[t];  DMA-xbar transpose -> delta[j, t];
  delta_w = w_red[j] * delta.
  one-hot (VE): MT[j,i] = (heads[j] == i) via is_equal vs iota row.
  MM3 (PE):  outT[t,i] += delta_w_j.T @ MT_j over j-tiles; += base[t]; DMA.
base[t] = sum(w_red)*tanh(b_comp)[t] + b_red and tanh(b_comp) are computed
on the host (keeps gpsimd's slow ext-isa partition reduce off device).
W is staged fp16 and loaded in 512-col pieces ordered by first use of the
strided t-sequence. Host transposes outT -> out per batch at gather time.
"""

import os
from contextlib import ExitStack

import numpy as np

import concourse.bass as bass
import concourse.tile as tile
from concourse import bacc, mybir
from concourse.bass_utils import run_bass_kernel_spmd

B, S, T = 8, 2048, 128
P = 128
N_CORES = 8
NST = S // P      # 16 s-tiles of 128
NSG = S // 512    # 4 s-groups of 512
F32 = mybir.dt.float32
F16 = mybir.dt.float16
I32 = mybir.dt.int32
AF = mybir.ActivationFunctionType
ALU = mybir.AluOpType

_NC_CACHE = {}


def build_nc():
    nc = bacc.Bacc("TRN2", target_bir_lowering=False, debug=False,
                   num_devices=N_CORES)

    tokT_d = nc.dram_tensor("tokT", [T, S], F16, kind="ExternalInput").ap()
    w_ptq_d = nc.dram_tensor("w_ptq", [P, T * T], F16, kind="ExternalInput").ap()
    bcompT_d = nc.dram_tensor("bcompT", [T, 1], F32, kind="ExternalInput").ap()
    wred_d = nc.dram_tensor("wred", [P, NST], F32, kind="ExternalInput").ap()
    heads_d = nc.dram_tensor("heads", [P, NST], I32, kind="ExternalInput").ap()
    basevT_d = nc.dram_tensor("basevT", [T, 1], F32,
                              kind="ExternalInput").ap()
    baseT_d = nc.dram_tensor("baseT", [T, 1], F32, kind="ExternalInput").ap()
    iota_d = nc.dram_tensor("iota", [1, S], F16, kind="ExternalInput").ap()
    outT_d = nc.dram_tensor("outT", [T, S], F32, kind="ExternalOutput").ap()

    with tile.TileContext(nc) as tc:
        _body(tc, tokT_d, w_ptq_d, bcompT_d, wred_d, heads_d, basevT_d,
              baseT_d, iota_d, outT_d)
    nc.compile()
    return nc


def _body(tc, tokT_d, w_ptq_d, bcompT_d, wred_d, heads_d, basevT_d, baseT_d,
          iota_d, outT_d):
    nc = tc.nc
    with ExitStack() as ctx:
        const = ctx.enter_context(tc.tile_pool(name="const", bufs=1))
        zpool = ctx.enter_context(tc.tile_pool(name="zpool", bufs=4))
        zbigp = ctx.enter_context(tc.tile_pool(name="zbigp", bufs=3))
        a16p = ctx.enter_context(tc.tile_pool(name="a16p", bufs=2))
        h6p = ctx.enter_context(tc.tile_pool(name="h6p", bufs=2))
        spool = ctx.enter_context(tc.tile_pool(name="spool", bufs=2))
        djp = ctx.enter_context(tc.tile_pool(name="djp", bufs=3))
        dwp = ctx.enter_context(tc.tile_pool(name="dwp", bufs=1))
        mtp = ctx.enter_context(tc.tile_pool(name="mtp", bufs=1))

        # ---- constants / inputs ----
        # Separate tiles per chunk so matmul deps release as each cast-DMA
        # lands (whole-tile deps would stall PE on the full 9MB load).
        # Queue order: tokT_0, W_0 first -> first MM1 starts after ~2 chunks.
        # staircase for 32-col-tiled reduction: Q32[:, 31] = 1, else 0;
        # E32_r = Q32[:, 31-r : 63-r] has its ones-column at position r.
        # Built first: MM2 needs it and gpsimd memsets queue behind any
        # DMA-descriptor work emitted earlier.
        Q32 = const.tile([P, 63], F16)
        nc.gpsimd.memset(Q32[:], 0.0)
        nc.gpsimd.memset(Q32[:, 31: 32], 1.0)

        tokTs = [const.tile([P, 512], F16, tag=f"tokT{g}", name=f"tokT{g}")
                 for g in range(NSG)]
        w_tiles = [const.tile([P, 2048], F16, tag=f"w{wc}", name=f"w{wc}")
                   for wc in range(8)]

        def load_w_pieces(wcs, piece, eng):
            ps = slice(512 * piece, 512 * (piece + 1))
            for wc in wcs:
                eng.dma_start(
                    out=w_tiles[wc][:, ps],
                    in_=w_ptq_d[:, 2048 * wc + 512 * piece:
                                2048 * wc + 512 * (piece + 1)])

        # Load order: tokT0 + the W pieces the first m-quads need lead the
        # sync (HWDGE) queue so compute starts without waiting on SWDGE
        # descriptor generation; the rest streams on the gpsimd queue in
        # strided-need order (tiles {0,2,4,6} serve t%32 in [0,16),
        # {1,3,5,7} serve m 16-31).
        nc.gpsimd.dma_start(out=tokTs[0][:], in_=tokT_d[:, 0:512])
        load_w_pieces((0, 2, 4, 6), 0, nc.gpsimd)
        for g in range(1, NSG):
            nc.gpsimd.dma_start(out=tokTs[g][:],
                                in_=tokT_d[:, 512 * g: 512 * (g + 1)])
        for piece in range(1, 4):
            load_w_pieces((0, 2, 4, 6), piece, nc.gpsimd)
        for piece in range(4):
            load_w_pieces((1, 3, 5, 7), piece, nc.gpsimd)
        # hT2 tanh builds are deferred into the main loop (start of each
        # group) so they don't block the ScalarE queue at startup.
        hT2s = [const.tile([P, 1024], F16, tag=f"hT2_{g}", name=f"hT2_{g}")
                for g in range(NSG)]
        iota_sb = const.tile([P, S], F16)
        nc.sync.dma_start(out=iota_sb[:], in_=iota_d[0:1, :].to_broadcast((P, S)))
        wred_sb = const.tile([P, NST], F32)
        nc.sync.dma_start(out=wred_sb[:], in_=wred_d[:])
        heads_sb = const.tile([P, NST], I32)
        nc.sync.dma_start(out=heads_sb[:], in_=heads_d[:])
        headsF = const.tile([P, NST], F32)
        nc.vector.tensor_copy(headsF[:], heads_sb[:])
        bcompT_sb = const.tile([T, 1], F32)
        nc.sync.dma_start(out=bcompT_sb[:], in_=bcompT_d[:])
        # basev = tanh(b_comp) and baseT = sum(w_red)*basev + b_red are
        # host-precomputed: keeps the slow gpsimd partition_all_reduce (and
        # its ~10us ext-isa library load) off the critical path.
        basevT = const.tile([T, 1], F32)
        nc.sync.dma_start(out=basevT[:], in_=basevT_d[:])
        baseT = const.tile([P, 1], F32)
        nc.sync.dma_start(out=baseT[:], in_=baseT_d[:])



        # ---- main loop: s-groups of 512, t processed in strided pairs
        # (m, m+32) / (m+64, m+96) so the reduction matmuls of consecutive
        # Z tiles land in different 32-wide PE column groups and execute
        # concurrently (4-way col tiling of the PE array). ----
        dws = []
        with tc.tile_pool(name="psumA", bufs=3, space="PSUM") as psumA, \
             tc.tile_pool(name="psumV", bufs=2, space="PSUM") as psumV:
            LAG = 7  # MM2 trails MM1 by LAG tps so PE never waits on Z
            TPG = T // 2  # tps per s-group
            NTP = NSG * TPG
            mm2q = []
            Vs = [None] * NSG
            hT6s = [None] * NSG

            def emit_group_tail(g):
                # after the last MM2 of group g: tanh, delta, transpose, scale
                actT = spool.tile([P, 512], F16, tag="actT", name="actT")
                nc.scalar.activation(actT[:], Vs[g][:], AF.Tanh,
                                     bias=bcompT_sb[:])
                dT = spool.tile([P, 512], F16, tag="dT", name="dT")
                nc.vector.tensor_scalar_sub(dT[:], actT[:], basevT[:])
                for k in range(4):
                    j = 4 * g + k
                    dj = djp.tile([P, P], F16, tag="dj", name="dj")
                    nc.sync.dma_start_transpose(out=dj[:],
                                                in_=dT[:, P * k: P * (k + 1)])
                    dw_j = dwp.tile([P, P], F16, tag=f"dw{j}", name=f"dw{j}")
                    nc.vector.tensor_scalar_mul(dw_j[:], dj[:],
                                                wred_sb[:, j: j + 1])
                    dws.append(dw_j)

            for gtp in range(NTP + LAG + 1):
                if gtp < NTP:
                    g, tp = gtp // TPG, gtp % TPG
                    if tp == 0:
                        hT2 = hT2s[g]
                        nc.scalar.activation(hT2[:, 0:512], tokTs[g][:],
                                             AF.Tanh)
                        nc.scalar.activation(hT2[:, 512:1024], tokTs[g][:],
                                             AF.Tanh)
                        # hT replicated x6 so one FD=3072 DVE multiply
                        # covers three t-pairs.
                        hT6 = h6p.tile([P, 3072], F16, tag="hT6", name="hT6")
                        for rep in range(3):
                            nc.vector.tensor_copy(
                                hT6[:, 1024 * rep: 1024 * (rep + 1)],
                                hT2[:])
                        hT6s[g] = hT6
                        Vs[g] = psumV.tile([P, 512], F32, space="PSUM",
                                           tag="V", name="V")
                    m, half = tp // 2, tp % 2
                    t0 = m + 64 * half
                    t1 = t0 + 32
                    A = psumA.tile([P, 1024], F32, space="PSUM", tag="A",
                                   name="A")
                    w_t0 = w_tiles[t0 // 16][:,
                                             T * (t0 % 16): T * (t0 % 16 + 1)]
                    w_t1 = w_tiles[t1 // 16][:,
                                             T * (t1 % 16): T * (t1 % 16 + 1)]
                    nc.tensor.matmul(A[:, 0:512], lhsT=w_t0,
                                     rhs=tokTs[g][:], start=True, stop=True)
                    nc.tensor.matmul(A[:, 512:1024], lhsT=w_t1,
                                     rhs=tokTs[g][:], start=True, stop=True)
                    c = tp % 4
                    if c != 3:
                        # ScalarE downcast to fp16 SBUF (2x-packed DVE); the
                        # three downcasts of a cycle land in one A16 tile so
                        # a single FD=3072 DVE multiply covers all three
                        # t-pairs (amortizes the fixed per-op DVE cost).
                        if c == 0:
                            A16 = a16p.tile([P, 3072], F16, tag="A16",
                                            name="A16")
                            Zb = zbigp.tile([P, 3072], F16, tag="Zb",
                                            name="Zb")
                        nc.scalar.activation(A16[:, 1024 * c: 1024 * (c + 1)],
                                             A[:], AF.Copy)
                        if c == 2:
                            nc.vector.tensor_tensor(out=Zb[:], in0=A16[:],
                                                    in1=hT6s[g][:],
                                                    op=ALU.mult)
                        mm2q.append((g, m, t0, t1, Zb, 1024 * c))
                    else:
                        Z = zpool.tile([P, 1024], F16, tag="Z", name="Z")
                        nc.vector.tensor_tensor(out=Z[:], in0=A[:],
                                                in1=hT2s[g][:], op=ALU.mult)
                        mm2q.append((g, m, t0, t1, Z, 0))
                # 32-col-tiled q-reduction: row r of col group j gets
                # sum_q Z_t with t = 32j + r. Emit as full quads (all 4
                # col groups back-to-back) so the four matmuls execute
                # concurrently in one PE slot. The lag queue is global so
                # the pipeline never refills at s-group boundaries.
                if gtp > LAG and (gtp - LAG) % 2 == 1:
                    for k in (gtp - LAG - 1, gtp - LAG):
                        g, m, t0, t1, Z, off = mm2q[k]
                        for t, zs in ((t0, slice(off, off + 512)),
                                      (t1, slice(off + 512, off + 1024))):
                            j, r = t // 32, t % 32
                            nc.tensor.matmul(Vs[g][32 * j: 32 * (j + 1), :],
                                             lhsT=Q32[:, 31 - r: 63 - r],
                                             rhs=Z[:, zs],
                                             start=(m == 0),
                                             stop=(m == 31),
                                             tile_position=(0, 32 * j),
                                             skip_group_check=True)
                        if m == 31 and t1 == 127:
                            emit_group_tail(g)

        # ---- one-hot scatter matrices: MT[j, i] = (iota[i] == head[j]).
        # Built late so they fill DVE slack near the main loop's tail. ----
        mts = []
        for j in range(NST):
            mt_j = mtp.tile([P, S], F16, tag=f"mt{j}", name=f"mt{j}")
            nc.vector.tensor_scalar(out=mt_j[:], in0=iota_sb[:],
                                    scalar1=headsF[:, j: j + 1], scalar2=None,
                                    op0=ALU.is_equal)
            mts.append(mt_j)

        # ---- scatter: outT[t, i] = sum_j delta_w[j, t] * MT[j, i] + base ----
        # c-outer so each output chunk's add + store overlaps the next
        # chunk's matmuls.
        outT_sb = const.tile([P, S], F32)
        with tc.tile_pool(name="psumO", bufs=1, space="PSUM") as psumO:
            for c in range(4):
                OT = psumO.tile([P, 512], F32, space="PSUM", tag=f"OT{c}",
                                name=f"OT{c}")
                for j in range(NST):
                    nc.tensor.matmul(OT[:], lhsT=dws[j][:],
                                     rhs=mts[j][:, 512 * c: 512 * (c + 1)],
                                     start=(j == 0), stop=(j == NST - 1))
                cs = slice(512 * c, 512 * (c + 1))
                nc.vector.tensor_scalar_add(outT_sb[:, cs], OT[:], baseT[:])
                nc.sync.dma_start(out=outT_d[:, cs], in_=outT_sb[:, cs])


def _prep_inputs(token_embeddings, dep_heads, W_comp, b_comp, w_red, b_red):
    """Host-side sharding + layout prep. One in_map per core (= per batch)."""
    token = np.asarray(token_embeddings, np.float32).astype(np.float16)
    heads = np.asarray(dep_heads, np.int32)
    W = np.asarray(W_comp, np.float32).astype(np.float16)
    w_ptq = np.ascontiguousarray(W.transpose(1, 0, 2).reshape(P, T * T))
    bcomp = np.asarray(b_comp, np.float32)
    bcompT = np.ascontiguousarray(bcomp.reshape(T, 1))
    wred_flat = np.asarray(w_red, np.float32).reshape(S)
    wred = np.ascontiguousarray(wred_flat.reshape(NST, P).T)
    bred = float(np.asarray(b_red, np.float32).reshape(1)[0])
    basev = np.tanh(bcomp.astype(np.float64))
    baseT = (wred_flat.astype(np.float64).sum() * basev + bred)
    basevT = np.ascontiguousarray(basev.astype(np.float32).reshape(T, 1))
    baseT = np.ascontiguousarray(baseT.astype(np.float32).reshape(T, 1))
    iota = np.arange(S, dtype=np.float16).reshape(1, S)

    in_maps = []
    for b in range(B):
        in_maps.append({
            "tokT": np.ascontiguousarray(token[b].T),
            "w_ptq": w_ptq,
            "bcompT": bcompT,
            "wred": wred,
            "heads": np.ascontiguousarray(heads[b].reshape(NST, P).T),
            "basevT": basevT,
            "baseT": baseT,
            "iota": iota,
        })
    return in_maps


def kernel(**inputs):
    if "nc" not in _NC_CACHE:
        _NC_CACHE["nc"] = build_nc()
    nc = _NC_CACHE["nc"]
    in_maps = _prep_inputs(
        inputs["token_embeddings"], inputs["dep_heads"], inputs["W_comp"],
        inputs["b_comp"], inputs["w_red"], inputs["b_red"])
    res = run_bass_kernel_spmd(nc, in_maps, core_ids=list(range(N_CORES)))
    out = np.empty((B, S, T), np.float32)
    for b in range(B):
        out[b] = res.results[b]["outT"].T
    return out

